# revision 1
# baseline (speedup 1.0000x reference)
"""Trainium2 Bass kernel for nn_Attention_48095043781121.

Math (reference):
    q,k,v = x@Wq, x@Wk, x@Wv          (per head h: columns [64h, 64h+64))
    A     = softmax_j(q.k^T / 8)
    p     = relu(pos@Wp1+bp1)@Wp2+bp2
    P[b,h,i,j] = softmax_j(ph[b,i,h] - ph[b,j,h] + bh[h])   with ph = p@Wh
               = softmax_j(-ph[b,j,h])          (shift-invariant in i, bh)
               = w[b,h,j]                        (independent of i!)
    attn  = ((1-g)A + gP) / rowsum               rowsum == 1 exactly
    out   = attn @ v ;  y = concat_heads(out) @ Wo + bo

So per (b,h):  y-contribution = (1-g_h) * (E @ v_h) / r  +  g_h * (w @ v_h)
with E = exp(S/8) (unnormalized scores), r[i] = sum_j E[i,j].

Sharding: 8 cores = 4 batches x 2 head-groups (heads 0-3 / 4-7).
Each core computes a partial y for its batch using its 4 heads and the
matching 256-column slice of Wq/Wk/Wv and 256-row slice of Wo; the host
sums the two partials per batch (tensor-parallel out-projection).

On-chip layout: feature-major ("transposed") activations everywhere the
contraction needs it; v is produced token-major directly.  The E@v
matmul appends a column holding 1/(1-g_h) so the same PSUM accumulation
yields r[i]/(1-g_h), whose reciprocal is the per-row output scale.
"""

import numpy as np
from contextlib import ExitStack

B, S, DIM, H, DH = 4, 512, 512, 8, 64
POS_DIM, PD8 = 3, 64
NCORES = 8
HGH = 4          # heads per head-group (per core)
HGF = HGH * DH   # feature columns per head-group = 256
KT = DIM // 128  # contraction tiles over model dim = 4
MT = HGF // 128  # feature tiles per head-group = 2
ST = S // 128    # token tiles = 4
DHP = DH + 2     # v columns padded: [v(64) | 1/(1-g) | 0]

# float32r streams 1 row/cycle (vs 4 for float32) when the moving free
# dim is >= 256 (TF32-like precision, ~1.5e-4 rel on a K=512 matmul).
USE_F32R = True

_CACHE = {}


def _build_program():
    import concourse.bass as bass
    import concourse.mybir as mybir
    import concourse.tile as tile
    from concourse import bacc
    from concourse.masks import make_identity

    F32 = mybir.dt.float32
    F16 = mybir.dt.float16
    MMD = mybir.dt.float32r if USE_F32R else F32
    AF = mybir.ActivationFunctionType
    ALU = mybir.AluOpType

    nc = bacc.Bacc(trn_type="TRN2", target_bir_lowering=False, debug=False)

    xT_d = nc.dram_tensor("xT", [128, KT * S], MMD, kind="ExternalInput")
    wq_d = nc.dram_tensor("Wq", [128, KT * HGF], MMD, kind="ExternalInput")
    wk_d = nc.dram_tensor("Wk", [128, KT * HGF], MMD, kind="ExternalInput")
    wv_d = nc.dram_tensor("Wv", [128, KT * HGF], MMD, kind="ExternalInput")
    wo_d = nc.dram_tensor("Wo", [128, MT * DIM], F16, kind="ExternalInput")
    bo_d = nc.dram_tensor("bo", [DIM], F16, kind="ExternalInput")
    posT_d = nc.dram_tensor("posT", [POS_DIM, S], MMD, kind="ExternalInput")
    wp1_d = nc.dram_tensor("Wp1", [POS_DIM, 4], MMD, kind="ExternalInput")
    bp1_d = nc.dram_tensor("bp1", [4], F32, kind="ExternalInput")
    wp2_d = nc.dram_tensor("Wp2", [POS_DIM, PD8], MMD, kind="ExternalInput")
    bp2_d = nc.dram_tensor("bp2", [PD8], F32, kind="ExternalInput")
    wh_d = nc.dram_tensor("Wh", [PD8, HGH], MMD, kind="ExternalInput")
    gate_d = nc.dram_tensor("gate", [HGH], F32, kind="ExternalInput")
    y_d = nc.dram_tensor("y", [S, DIM], F32, kind="ExternalOutput")

    with tile.TileContext(nc) as tc, ExitStack() as ctx:
        sing = ctx.enter_context(tc.tile_pool(name="sing", bufs=1))
        epool = ctx.enter_context(tc.tile_pool(name="epool", bufs=2))
        ypool = ctx.enter_context(tc.tile_pool(name="ypool", bufs=2))
        scpool = ctx.enter_context(tc.tile_pool(name="scpool", bufs=4))
        ps_mm = ctx.enter_context(tc.tile_pool(name="ps_mm", bufs=3, space="PSUM"))
        ps_u = ctx.enter_context(tc.tile_pool(name="ps_u", bufs=3, space="PSUM"))
        ps_t = ctx.enter_context(tc.tile_pool(name="ps_t", bufs=2, space="PSUM"))

        # ---- constants + PE warm-up (runs during the input DMA head;
        # a ~4.5us burst of back-to-back matmuls flips the HAM clock
        # gate to 8/8 before the real matmuls begin)
        ident = sing.tile([128, 128], F32)
        make_identity(nc, ident)
        ones1_f = sing.tile([1, 128], F32)
        nc.vector.memset(ones1_f, 1.0)
        ones1 = sing.tile([1, 128], MMD)
        nc.vector.tensor_copy(ones1, ones1_f)
        ones1_h = sing.tile([1, 128], F16)
        nc.vector.tensor_copy(ones1_h, ones1_f)
        ident_r = sing.tile([128, 128], MMD)
        nc.vector.tensor_copy(ident_r, ident)
        warm_f = sing.tile([128, 512], F32)
        nc.vector.memset(warm_f, 0.5)
        warm_src = sing.tile([128, 512], MMD)
        nc.vector.tensor_copy(warm_src, warm_f)
        with nc.named_scope("warmup"):
            # K=128 and N=512 (1 cyc/row) -- the HAM clock gate tracks
            # PSUM drain duty, so N<256 fp32r (4 cyc/row) and K=1 bursts
            # never flip it to 8/8.  Sized to bridge the input-DMA head.
            for _ in range(35):
                wps = ps_mm.tile([128, 512], F32, tag="mm")
                nc.tensor.matmul(wps, ident_r, warm_src, start=True, stop=True)

        # ---------------- input DMAs ----------------
        # Split / ordered so the first projection matmuls can start
        # early; issues are spread across engine queues.
        xT = sing.tile([128, KT, S], MMD)
        wq = sing.tile([128, KT, HGF], MMD)
        wk = sing.tile([128, KT, HGF], MMD)
        wv = sing.tile([128, KT, HGF], MMD)
        wo = sing.tile([128, MT, DIM], F16)
        xT_r = xT_d.ap()
        wq_r = wq_d.ap()
        wk_r = wk_d.ap()
        wv_r = wv_d.ap()
        nc.sync.dma_start(out=xT[:, 0:2, :], in_=xT_r[:, 0 : 2 * S])
        nc.sync.dma_start(out=wk, in_=wk_r)
        nc.sync.dma_start(out=wv, in_=wv_r)
        # small pos-path inputs on the scalar queue
        posT = sing.tile([POS_DIM, S], MMD)
        nc.scalar.dma_start(out=posT, in_=posT_d.ap())
        wp1 = sing.tile([POS_DIM, 4], MMD)
        nc.scalar.dma_start(out=wp1, in_=wp1_d.ap())
        bp1 = sing.tile([4, 1], F32)
        nc.scalar.dma_start(out=bp1, in_=bp1_d.ap()[:, None])
        wp2 = sing.tile([POS_DIM, PD8], MMD)
        nc.scalar.dma_start(out=wp2, in_=wp2_d.ap())
        bp2 = sing.tile([PD8, 1], F32)
        nc.scalar.dma_start(out=bp2, in_=bp2_d.ap()[:, None])
        wh = sing.tile([PD8, HGH], MMD)
        nc.scalar.dma_start(out=wh, in_=wh_d.ap())
        gate_c = sing.tile([HGH, 1], F32)
        nc.scalar.dma_start(out=gate_c, in_=gate_d.ap()[:, None])
        gate_r = sing.tile([1, HGH], F32)
        nc.scalar.dma_start(out=gate_r, in_=gate_d.ap()[None, :])
        nc.scalar.dma_start(out=xT[:, 2:KT, :], in_=xT_r[:, 2 * S : KT * S])
        nc.scalar.dma_start(out=wq, in_=wq_r)
        nc.scalar.dma_start(out=wo, in_=wo_d.ap())
        bo_sb = sing.tile([1, DIM], F16)
        nc.scalar.dma_start(out=bo_sb, in_=bo_d.ap()[None, :])

        # ---- gate -> 1/(1-g) -> v_aug pad columns.  The partition
        # broadcast is a K=1 matmul with a ones stationary vector (the
        # DMA broadcast alternative costs ~5us per 128-partition fill).
        v_aug = sing.tile([128, ST, HGH, DHP], F16)
        with nc.named_scope("gate_fill"):
            g_c = sing.tile([HGH, 1], F32)
            nc.scalar.activation(g_c, gate_c, AF.Sigmoid)
            g_r = sing.tile([1, HGH], F32)
            nc.scalar.activation(g_r, gate_r, AF.Sigmoid)
            one_mg_r = sing.tile([1, HGH], F32)
            nc.vector.tensor_scalar(one_mg_r, g_r, -1.0, 1.0, ALU.mult, ALU.add)
            inv_r = sing.tile([1, HGH], F32)
            nc.vector.reciprocal(inv_r, one_mg_r)
            i2row = sing.tile([1, 2 * HGH], F32)
            nc.vector.memset(i2row, 0.0)
            nc.vector.tensor_copy(
                i2row.rearrange("p (c two) -> p c two", two=2)[:, :, 0], inv_r
            )
            iv_ps = ps_t.tile([128, 2 * HGH], F32, tag="t")
            nc.tensor.matmul(iv_ps, ones1_f, i2row, start=True, stop=True)
            for tt in range(ST):
                nc.vector.tensor_copy(v_aug[:, tt, :, DH : DH + 2], iv_ps)

        # ---- position MLP (tiny; PE work hides under the DMA head)
        with nc.named_scope("pos_path"):
            p1ps = ps_mm.tile([4, S], F32, tag="mm")
            nc.tensor.matmul(p1ps, wp1, posT, start=True, stop=True)
            p1 = sing.tile([4, S], MMD)
            nc.scalar.activation(p1, p1ps, AF.Relu, bias=bp1)
            p2ps = ps_mm.tile([PD8, S], F32, tag="mm")
            nc.tensor.matmul(p2ps, wp2, p1[0:POS_DIM, :], start=True, stop=True)
            p2 = sing.tile([PD8, S], MMD)
            nc.scalar.activation(p2, p2ps, AF.Identity, bias=bp2)
            phps = ps_mm.tile([HGH, S], F32, tag="mm")
            nc.tensor.matmul(phps, wh, p2, start=True, stop=True)
            wexp = sing.tile([HGH, S], F32)
            wsum = sing.tile([HGH, 1], F32)
            nc.scalar.activation(wexp, phps, AF.Exp, scale=-1.0, accum_out=wsum)
            winv = sing.tile([HGH, 1], F32)
            nc.vector.reciprocal(winv, wsum)
            # fold the gate in now: w_g rows sum to g_h instead of 1
            gwin = sing.tile([HGH, 1], F32)
            nc.vector.tensor_mul(gwin, winv, g_c)
            w_sb = sing.tile([HGH, S], F32)
            nc.vector.tensor_scalar_mul(w_sb, wexp, gwin)

        # ---------------- projections ----------------
        kT_sb = sing.tile([128, MT, S], F16)
        qT_sb = sing.tile([128, MT, S], F16)
        with nc.named_scope("proj_kq"):
            for m in range(MT):
                for dst, w in ((kT_sb, wk), (qT_sb, wq)):
                    ps = ps_mm.tile([128, S], F32, tag="mm")
                    for kk in range(KT):
                        nc.tensor.matmul(
                            ps,
                            w[:, kk, 128 * m : 128 * (m + 1)],
                            xT[:, kk, :],
                            start=(kk == 0),
                            stop=(kk == KT - 1),
                        )
                    nc.vector.tensor_copy(dst[:, m, :], ps)

        with nc.named_scope("proj_v"):
            for tt in range(ST):
                ps = ps_mm.tile([128, HGF], F32, tag="mm")
                for kk in range(KT):
                    nc.tensor.matmul(
                        ps,
                        xT[:, kk, 128 * tt : 128 * (tt + 1)],
                        wv[:, kk, :],
                        start=(kk == 0),
                        stop=(kk == KT - 1),
                    )
                nc.vector.tensor_copy(
                    v_aug[:, tt, :, 0:DH],
                    ps.rearrange("p (h c) -> p h c", c=DH),
                )

        # ---- g*WV broadcast tiles.  Each head's g_h*(w@v_h) row is
        # produced on partition 0 by an M=1 fp32 matmul, then broadcast
        # to 128 partitions with a K=1 ones matmul.
        with nc.named_scope("gwv"):
            wj = sing.tile([128, ST, HGH], F16)
            for jt in range(ST):
                wt = ps_t.tile([128, HGH], F32, tag="t")
                nc.tensor.transpose(
                    wt, w_sb[:, 128 * jt : 128 * (jt + 1)], ident[0:HGH, 0:HGH]
                )
                nc.vector.tensor_copy(wj[:, jt, :], wt)
            gwv_rows = sing.tile([1, HGF], F32)
            for hl in range(HGH):
                wvp = ps_u.tile([1, DH], F32, tag="u")
                for jt in range(ST):
                    nc.tensor.matmul(
                        wvp,
                        wj[:, jt, hl : hl + 1],
                        v_aug[:, jt, hl, 0:DH],
                        start=(jt == 0),
                        stop=(jt == ST - 1),
                    )
                nc.vector.tensor_copy(gwv_rows[:, DH * hl : DH * (hl + 1)], wvp)
            GWV_ps = ps_mm.tile([128, HGF], F32, tag="mm")
            nc.tensor.matmul(GWV_ps, ones1_f, gwv_rows, start=True, stop=True)
            GWV = sing.tile([128, HGF], F32)
            nc.vector.tensor_copy(GWV, GWV_ps)

        # ---------------- attention ----------------
        ocat = sing.tile([128, ST, HGF], F32)
        oT = sing.tile([128, MT, S], F16)
        e_tiles = []
        for m in range(MT):
            with nc.named_scope(f"score_p{m}"):
                e_sb = epool.tile([128, 2, ST, S], F16, tag="e")
                e_tiles.append(e_sb)
                for jt in range(ST):
                    for sub in range(2):
                        off = 64 * sub
                        sps = ps_mm.tile([128, S], F32, tag="mm")
                        nc.tensor.matmul(
                            sps,
                            kT_sb[off : off + 64, m, 128 * jt : 128 * (jt + 1)],
                            qT_sb[off : off + 64, m, :],
                            start=True,
                            stop=True,
                        )
                        nc.scalar.activation(
                            e_sb[:, sub, jt, :], sps, AF.Exp, scale=0.125
                        )
        # pair 0: all heads' E@v, then transpose its 128 ocat columns.
        # pair 1: per query-tile, finish both heads, transpose, and emit
        # that tile's out-projection + y DMA immediately -- the output
        # stream drains DURING the attention window instead of after it.
        def mm2_one(m, sub, it):
            e_sb = e_tiles[m]
            hl = 2 * m + sub
            ups = ps_u.tile([128, DHP], F32, tag="u")
            for jt in range(ST):
                nc.tensor.matmul(
                    ups,
                    e_sb[:, sub, jt, 128 * it : 128 * (it + 1)],
                    v_aug[:, jt, hl, :],
                    start=(jt == 0),
                    stop=(jt == ST - 1),
                )
            # ups[:, :64] = E@v_h ;  ups[:, 64] = r/(1-g_h)
            sc = scpool.tile([128, 1], F32, tag="sc")
            nc.vector.reciprocal(sc, ups[:, DH : DH + 1])
            dst = ocat[:, it, DH * hl : DH * (hl + 1)]
            nc.vector.scalar_tensor_tensor(
                dst,
                ups[:, 0:DH],
                sc,
                GWV[:, DH * hl : DH * (hl + 1)],
                ALU.mult,
                ALU.add,
            )

        def transpose_pair(m, it):
            tp = ps_t.tile([128, 128], F32, tag="t")
            nc.tensor.transpose(tp, ocat[:, it, 128 * m : 128 * (m + 1)], ident)
            nc.vector.tensor_copy(oT[:, m, 128 * it : 128 * (it + 1)], tp)

        with nc.named_scope("attn_p0"):
            for sub in range(2):
                for it in range(ST):
                    mm2_one(0, sub, it)
            for it in range(ST):
                transpose_pair(0, it)

        with nc.named_scope("attn_p1_outproj"):
            for it in range(ST):
                mm2_one(1, 0, it)
                mm2_one(1, 1, it)
                transpose_pair(1, it)
                yps = ps_mm.tile([128, DIM], F32, tag="mm")
                for fm in range(MT):
                    nc.tensor.matmul(
                        yps,
                        oT[:, fm, 128 * it : 128 * (it + 1)],
                        wo[:, fm, :],
                        start=(fm == 0),
                        stop=False,
                    )
                nc.tensor.matmul(yps, ones1_h, bo_sb, start=False, stop=True)
                ysb = ypool.tile([128, DIM], F32, tag="y")
                nc.vector.tensor_copy(ysb, yps)
                nc.sync.dma_start(
                    out=y_d.ap()[128 * it : 128 * (it + 1), :], in_=ysb
                )

    nc.compile()
    return nc


def _get_program():
    if "nc" not in _CACHE:
        _CACHE["nc"] = _build_program()
    return _CACHE["nc"]


def _ktile(a, dtype=np.float32):
    # [K*128, n] -> [128, K*n] (per-partition-contiguous k-tile layout)
    k = a.shape[0] // 128
    return np.ascontiguousarray(
        a.reshape(k, 128, a.shape[1]).transpose(1, 0, 2).reshape(128, -1).astype(dtype)
    )


def _make_in_maps(inputs):
    f = lambda a: np.ascontiguousarray(np.asarray(a), dtype=np.float32)
    x = f(inputs["x"])
    pos = f(inputs["pos"])
    Wq, Wk, Wv, Wo = f(inputs["Wq"]), f(inputs["Wk"]), f(inputs["Wv"]), f(inputs["Wo"])
    bo = f(inputs["bo"])
    Wp1, bp1, Wp2, bp2 = f(inputs["Wp1"]), f(inputs["bp1"]), f(inputs["Wp2"]), f(inputs["bp2"])
    Wh, gate = f(inputs["Wh"]), f(inputs["gate"])
    # pad the tiny pos-MLP first layer to 4 outputs (fp32r even-size rule)
    Wp1 = np.concatenate([Wp1, np.zeros((POS_DIM, 1), np.float32)], axis=1)
    bp1 = np.concatenate([bp1, np.zeros(1, np.float32)])

    in_maps = []
    for c in range(NCORES):
        b, hg = c // 2, c % 2
        cs = slice(HGF * hg, HGF * (hg + 1))
        in_maps.append(
            {
                "xT": _ktile(x[b].T),
                "Wq": _ktile(Wq[:, cs]),
                "Wk": _ktile(Wk[:, cs]),
                "Wv": _ktile(Wv[:, cs]),
                "Wo": _ktile(Wo[cs, :], np.float16),
                "bo": (bo if hg == 0 else np.zeros_like(bo)).astype(np.float16),
                "posT": np.ascontiguousarray(pos[b].T),
                "Wp1": Wp1,
                "bp1": bp1,
                "Wp2": Wp2,
                "bp2": bp2,
                "Wh": np.ascontiguousarray(Wh[:, HGH * hg : HGH * (hg + 1)]),
                "gate": np.ascontiguousarray(gate[HGH * hg : HGH * (hg + 1)]),
            }
        )
    return in_maps


def run(inputs, trace=False):
    """Run on 8 NeuronCores; returns (out [B,S,DIM] fp32, BassKernelResults)."""
    from concourse.bass_utils import run_bass_kernel_spmd

    nc = _get_program()
    in_maps = _make_in_maps(inputs)
    res = run_bass_kernel_spmd(
        nc, in_maps, core_ids=list(range(NCORES)), trace=trace
    )
    out = np.empty((B, S, DIM), np.float32)
    for b in range(B):
        out[b] = res.results[2 * b]["y"] + res.results[2 * b + 1]["y"]
    return out, res


def kernel(**inputs):
    out, _ = run(inputs, trace=False)
    return out

